# revision 1
# baseline (speedup 1.0000x reference)
import sys
import numpy as np

sys.path.insert(0, "/opt/trn_rl_repo")

N_CORES = 8
B_FULL, C, H, W = 64, 128, 80, 80
BL = B_FULL // N_CORES  # batches per core
HW = H * W  # 6400
NH, HD = 4, 32
SIG2 = 2.0 * 0.3**2

_CACHE = {}


def _consts():
    import jax
    import jax.numpy as jnp

    key = jax.random.key(0)
    ks = jax.random.split(key, 12)

    def lin(k, o, i):
        return np.asarray(jax.random.normal(k, (o, i), jnp.float32)) / np.sqrt(i)

    wq, wk, wv, wo = lin(ks[2], 128, 128), lin(ks[3], 128, 128), lin(ks[4], 128, 128), lin(ks[5], 128, 128)
    w1, w2 = lin(ks[6], 256, 128), lin(ks[7], 128, 256)
    return wq, wk, wv, wo, w1, w2


def _build():
    import concourse.bass as bass
    import concourse.tile as tile
    from concourse import bacc, mybir

    f32 = mybir.dt.float32
    f32r = mybir.dt.float32r
    AF = mybir.ActivationFunctionType
    OP = mybir.AluOpType
    AX = mybir.AxisListType

    nc = bacc.Bacc("TRN2", target_bir_lowering=False, debug=False)
    d = {}
    d["x"] = nc.dram_tensor("x", [BL, C, H, W], f32, kind="ExternalInput").ap()
    d["masks"] = nc.dram_tensor("masks", [BL, 2, 640, 640], f32, kind="ExternalInput").ap()
    for nm, shp in [
        ("wqt", [128, 128]), ("wkt", [128, 128]), ("wvt", [128, 128]), ("wot", [128, 128]),
        ("w1t", [128, 256]), ("w2t0", [128, 128]), ("w2t1", [128, 128]),
        ("bq", [128, 1]), ("bk", [128, 1]), ("bv", [128, 1]), ("bo", [128, 1]),
        ("b1", [128, 2]), ("b2", [128, 1]),
        ("g1", [128, 1]), ("be1", [128, 1]), ("g2", [128, 1]), ("be2", [128, 1]),
        ("ones80", [80, 1]), ("ones128", [128, 1]), ("onesr", [1, 128]), ("id128", [128, 128]),
        ("ys80", [80, 1]), ("xs2", [1, 160]), ("yg2", [2, 80]), ("xg2", [2, 80]), ("xgxg", [1, 160]),
        ("signs2", [2, 1]),
    ]:
        d[nm] = nc.dram_tensor(nm, shp, f32, kind="ExternalInput").ap()
    out_d = nc.dram_tensor("out", [BL, C, H, W], f32, kind="ExternalOutput").ap()

    with tile.TileContext(nc) as tc:
        from contextlib import ExitStack

        ctx = ExitStack()
        cpool = ctx.enter_context(tc.tile_pool(name="consts", bufs=1))
        xpool = ctx.enter_context(tc.tile_pool(name="x", bufs=3))
        mpool = ctx.enter_context(tc.tile_pool(name="mfull", bufs=2))
        spool = ctx.enter_context(tc.tile_pool(name="stat", bufs=2))
        tiny = ctx.enter_context(tc.tile_pool(name="tiny", bufs=4))
        hpool = ctx.enter_context(tc.tile_pool(name="hwx", bufs=1))
        wpool = ctx.enter_context(tc.tile_pool(name="wrep", bufs=2))
        ps_n = ctx.enter_context(tc.tile_pool(name="psn", bufs=2, space="PSUM"))
        ps_s = ctx.enter_context(tc.tile_pool(name="pss", bufs=3, space="PSUM"))
        ps_sp = ctx.enter_context(tc.tile_pool(name="pssp", bufs=3, space="PSUM"))
        dscr = ctx.enter_context(tc.tile_pool(name="dscr", bufs=2, space="DRAM"))

        cst = {}
        for nm in ["wqt", "wkt", "wvt", "wot", "w1t", "w2t0", "w2t1", "bq", "bk", "bv",
                   "bo", "b1", "b2", "g1", "be1", "g2", "be2", "ones80", "ones128",
                   "onesr", "id128", "ys80", "xs2", "yg2", "xg2", "xgxg", "signs2"]:
            t = cpool.tile(list(d[nm].shape), f32, tag=nm)
            nc.sync.dma_start(t[:], d[nm][:])
            cst[nm] = t

        for b in range(BL):
            # ---- load x[b] h-major: (h=80, c-major x 80w) ----
            X = xpool.tile([80, C * W], f32, tag="X")
            nc.sync.dma_start(X[:], d["x"][b].rearrange("c h w -> h c w"))

            # ---- masks: rows ::8 of (640,640), both n ----
            Mf = mpool.tile([80, 1280], f32, tag="Mf")
            for n in range(2):
                src = d["masks"][b, n].rearrange("(h s) w -> h s w", s=8)[:, 0, :]
                nc.sync.dma_start(Mf[:, n * 640:(n + 1) * 640], src)

            # STAT = [Msub(160) | Mbin(160) | Mby(160)]
            ST = spool.tile([80, 480], f32, tag="ST")
            for n in range(2):
                sub = Mf[:, n * 640:(n + 1) * 640].rearrange("h (w s) -> h w s", s=8)[:, :, 0]
                nc.vector.tensor_copy(out=ST[:, n * 80:(n + 1) * 80], in_=sub)
            nc.vector.tensor_scalar(out=ST[:, 160:320], in0=ST[:, 0:160], scalar1=0.5,
                                    scalar2=None, op0=OP.is_gt)
            nc.vector.tensor_scalar(out=ST[:, 320:480], in0=ST[:, 160:320],
                                    scalar1=cst["ys80"][:], scalar2=None, op0=OP.mult)
            ps_stat = ps_s.tile([1, 480], f32, tag="pss")
            nc.tensor.matmul(ps_stat[:], cst["ones80"][:], ST[:], start=True, stop=True)

            sm = tiny.tile([1, 16], f32, tag="sm")  # [msum0,1 cnt0,1 cym0,1 | cxm0,1 | rm0,1 rc0,1 | px0,1 py0,1]
            nc.vector.tensor_reduce(out=sm[:, 0:6], in_=ps_stat[:].rearrange("p (g w) -> p g w", w=80),
                                    axis=AX.X, op=OP.add)
            tx = tiny.tile([1, 160], f32, tag="tx")
            nc.vector.tensor_tensor(out=tx[:], in0=ps_stat[:, 160:320], in1=cst["xs2"][:], op=OP.mult)
            nc.vector.tensor_reduce(out=sm[:, 6:8], in_=tx[:].rearrange("p (g w) -> p g w", w=80),
                                    axis=AX.X, op=OP.add)
            nc.vector.tensor_scalar(out=sm[:, 8:10], in0=sm[:, 0:2], scalar1=1e-6, scalar2=None, op0=OP.add)
            nc.vector.reciprocal(out=sm[:, 8:10], in_=sm[:, 8:10])
            nc.vector.reciprocal(out=sm[:, 10:12], in_=sm[:, 2:4])
            # pos: px = cxm*rc*(2/W)-1 ; py = cym*rc*(2/H)-1
            pxy = tiny.tile([1, 4], f32, tag="pxy")
            nc.vector.tensor_tensor(out=pxy[:, 0:2], in0=sm[:, 6:8], in1=sm[:, 10:12], op=OP.mult)
            nc.vector.tensor_tensor(out=pxy[:, 2:4], in0=sm[:, 4:6], in1=sm[:, 10:12], op=OP.mult)
            nc.vector.tensor_scalar(out=pxy[:], in0=pxy[:], scalar1=2.0 / 80.0, scalar2=-1.0,
                                    op0=OP.mult, op1=OP.add)
            dd = tiny.tile([1, 4], f32, tag="dd")  # [dx dy | dsum d]
            g = pxy[:].rearrange("p (g i) -> p g i", i=2)
            nc.vector.tensor_tensor(out=dd[:, 0:2], in0=g[:, :, 0], in1=g[:, :, 1], op=OP.subtract)
            nc.vector.tensor_tensor(out=dd[:, 0:2], in0=dd[:, 0:2], in1=dd[:, 0:2], op=OP.mult)
            nc.vector.tensor_reduce(out=dd[:, 2:3], in_=dd[:, 0:2], axis=AX.X, op=OP.add)
            nc.scalar.activation(out=dd[:, 3:4], in_=dd[:, 2:3], func=AF.Sqrt)

            # ---- nodes: 80 accumulating matmuls over w, k=h ----
            ps_nodes = ps_n.tile([128, 2], f32, tag="ps_nodes")
            Xv = X[:].rearrange("h (c w) -> h c w", w=W)
            Mv = ST[:, 0:160].rearrange("h (n w) -> h n w", w=W)
            for w in range(W):
                nc.tensor.matmul(ps_nodes[:], Xv[:, :, w], Mv[:, :, w],
                                 start=(w == 0), stop=(w == W - 1))
            prms = ps_s.tile([128, 2], f32, tag="pss")
            nc.tensor.matmul(prms[:], cst["onesr"][:], sm[:, 8:10], start=True, stop=True)
            rms = tiny.tile([128, 2], f32, tag="rms")
            nc.scalar.activation(out=rms[:], in_=prms[:], func=AF.Copy)
            nodes = tiny.tile([128, 2], f32, tag="nodes")
            nc.vector.tensor_tensor(out=nodes[:], in0=ps_nodes[:], in1=rms[:], op=OP.mult)

            # ---- attention ----
            qkv = tiny.tile([128, 6], f32, tag="qkv")
            for i, (wt, bb) in enumerate([("wqt", "bq"), ("wkt", "bk"), ("wvt", "bv")]):
                pq = ps_s.tile([128, 2], f32, tag="pss")
                nc.tensor.matmul(pq[:], cst[wt][:], nodes[:], start=True, stop=True)
                nc.scalar.activation(out=qkv[:, 2 * i:2 * i + 2], in_=pq[:], func=AF.Identity,
                                     bias=cst[bb][:])
            ps_sc = ps_s.tile([2, 8], f32, tag="pss")
            qk3 = tiny.tile([32, 4], f32, tag="qk3")
            nc.gpsimd.dma_start(qk3[:], qkv[96:128, 0:4])
            for h in range(NH):
                if h < 3:
                    ql, kl = qkv[32 * h:32 * h + 32, 0:2], qkv[32 * h:32 * h + 32, 2:4]
                else:
                    ql, kl = qk3[:, 0:2], qk3[:, 2:4]
                nc.tensor.matmul(ps_sc[:, 2 * h:2 * h + 2], ql, kl, start=True, stop=True)
            sc_sb = tiny.tile([2, 8], f32, tag="sc_sb")
            nc.scalar.activation(out=sc_sb[:], in_=ps_sc[:], func=AF.Copy)
            sdiff = tiny.tile([2, 4], f32, tag="sdiff")
            scv = sc_sb[:].rearrange("p (h j) -> p h j", j=2)
            nc.vector.tensor_tensor(out=sdiff[:], in0=scv[:, :, 0], in1=scv[:, :, 1], op=OP.subtract)
            pdp = ps_s.tile([2, 1], f32, tag="pss")
            nc.tensor.matmul(pdp[:], cst["onesr"][:, 0:2], dd[:, 3:4], start=True, stop=True)
            dpm = tiny.tile([2, 1], f32, tag="dpm")
            nc.vector.tensor_scalar(out=dpm[:], in0=pdp[:], scalar1=cst["signs2"][:],
                                    scalar2=None, op0=OP.mult)
            a24 = tiny.tile([2, 4], f32, tag="a24")
            nc.scalar.activation(out=a24[:], in_=sdiff[:], func=AF.Sigmoid,
                                 scale=1.0 / np.sqrt(32.0), bias=dpm[:])
            adr = dscr.tile([8], f32, tag="adr")
            nc.gpsimd.dma_start(adr[:].rearrange("(h i) -> i h", i=2), a24[:])
            a128 = tiny.tile([128, 2], f32, tag="a128")
            asrc = adr[:].rearrange("(h r i) -> h r i", h=4, r=1, i=2).broadcast_to((4, 32, 2))
            nc.gpsimd.dma_start(a128[:], asrc)
            ao = tiny.tile([128, 2], f32, tag="ao")
            vd = tiny.tile([128, 1], f32, tag="vd")
            nc.vector.tensor_tensor(out=vd[:], in0=qkv[:, 4:5], in1=qkv[:, 5:6], op=OP.subtract)
            for i in range(2):
                nc.vector.tensor_scalar(out=ao[:, i:i + 1], in0=vd[:], scalar1=a128[:, i:i + 1],
                                        scalar2=qkv[:, 5:6], op0=OP.mult, op1=OP.add)

            def layernorm(res_in0, res_in1, gname, bname, tag):
                st = tiny.tile([128, 4], f32, tag=tag)
                nc.vector.tensor_tensor(out=st[:, 0:2], in0=res_in0, in1=res_in1, op=OP.add)
                nc.scalar.activation(out=st[:, 2:4], in_=st[:, 0:2], func=AF.Square)
                pl = ps_s.tile([1, 4], f32, tag="pss")
                nc.tensor.matmul(pl[:], cst["ones128"][:], st[:], start=True, stop=True)
                ms = tiny.tile([1, 4], f32, tag=tag + "m")  # [mean2 | rstd2]
                nc.vector.tensor_scalar(out=ms[:, 0:2], in0=pl[:, 0:2], scalar1=1.0 / 128,
                                        scalar2=None, op0=OP.mult)
                v = tiny.tile([1, 2], f32, tag=tag + "v")
                nc.vector.tensor_tensor(out=v[:], in0=ms[:, 0:2], in1=ms[:, 0:2], op=OP.mult)
                nc.vector.tensor_scalar(out=v[:], in0=v[:], scalar1=-1.0, scalar2=None, op0=OP.mult)
                nc.vector.tensor_scalar(out=ms[:, 2:4], in0=pl[:, 2:4], scalar1=1.0 / 128,
                                        scalar2=1e-5, op0=OP.mult, op1=OP.add)
                nc.vector.tensor_tensor(out=ms[:, 2:4], in0=ms[:, 2:4], in1=v[:], op=OP.add)
                nc.scalar.activation(out=ms[:, 2:4], in_=ms[:, 2:4], func=AF.Sqrt)
                nc.vector.reciprocal(out=ms[:, 2:4], in_=ms[:, 2:4])
                pmsr = ps_s.tile([128, 4], f32, tag="pss")
                nc.tensor.matmul(pmsr[:], cst["onesr"][:], ms[:], start=True, stop=True)
                msr = tiny.tile([128, 4], f32, tag=tag + "r")
                nc.scalar.activation(out=msr[:], in_=pmsr[:], func=AF.Copy)
                hh = tiny.tile([128, 2], f32, tag=tag + "h")
                nc.vector.tensor_tensor(out=hh[:], in0=st[:, 0:2], in1=msr[:, 0:2], op=OP.subtract)
                nc.vector.tensor_tensor(out=hh[:], in0=hh[:], in1=msr[:, 2:4], op=OP.mult)
                nc.vector.tensor_scalar(out=hh[:], in0=hh[:], scalar1=cst[gname][:],
                                        scalar2=cst[bname][:], op0=OP.mult, op1=OP.add)
                return hh

            po = ps_s.tile([128, 2], f32, tag="pss")
            nc.tensor.matmul(po[:], cst["wot"][:], ao[:], start=True, stop=True)
            y1 = tiny.tile([128, 2], f32, tag="y1")
            nc.scalar.activation(out=y1[:], in_=po[:], func=AF.Identity, bias=cst["bo"][:])
            h1 = layernorm(y1[:], nodes[:], "g1", "be1", "ln1")

            z = tiny.tile([128, 4], f32, tag="z")
            for i in range(2):
                pz = ps_s.tile([128, 2], f32, tag="pss")
                nc.tensor.matmul(pz[:], cst["w1t"][:, 128 * i:128 * i + 128], h1[:],
                                 start=True, stop=True)
                nc.scalar.activation(out=z[:, 2 * i:2 * i + 2], in_=pz[:], func=AF.Relu,
                                     bias=cst["b1"][:, i:i + 1])
            py2 = ps_s.tile([128, 2], f32, tag="pss")
            nc.tensor.matmul(py2[:], cst["w2t0"][:], z[:, 0:2], start=True, stop=False)
            nc.tensor.matmul(py2[:], cst["w2t1"][:], z[:, 2:4], start=False, stop=True)
            y2 = tiny.tile([128, 2], f32, tag="y2")
            nc.scalar.activation(out=y2[:], in_=py2[:], func=AF.Identity, bias=cst["b2"][:])
            h2 = layernorm(y2[:], h1[:], "g2", "be2", "ln2")

            # ---- splat vectors ----
            pxt = tiny.tile([2, 2], f32, tag="pxt")  # [:,0]=px_n [:,1]=py_n on partition n
            nc.gpsimd.dma_start(pxt[:, 0:1], pxy[:, 0:2])
            nc.gpsimd.dma_start(pxt[:, 1:2], pxy[:, 2:4])
            ty = tiny.tile([2, 160], f32, tag="tyx")  # [ty | tx]
            nc.vector.tensor_scalar(out=ty[:, 0:80], in0=cst["yg2"][:], scalar1=pxt[:, 1:2],
                                    scalar2=None, op0=OP.subtract)
            nc.vector.tensor_scalar(out=ty[:, 80:160], in0=cst["xg2"][:], scalar1=pxt[:, 0:1],
                                    scalar2=None, op0=OP.subtract)
            nc.vector.tensor_tensor(out=ty[:], in0=ty[:], in1=ty[:], op=OP.mult)
            wyr = hpool.tile([2, 80], f32r, tag="wyr")
            nc.scalar.activation(out=wyr[:], in_=ty[:, 0:80], func=AF.Exp, scale=-1.0 / SIG2)
            # single-partition wx row pair [wx0 | wx1] via free-step-0 TT read
            txr = tiny.tile([1, 160], f32, tag="txr")
            pxr = pxy[:, 0:2].broadcast_to((1, 2, 80))
            nc.vector.tensor_tensor(out=txr[:], in0=cst["xgxg"][:], in1=pxr, op=OP.subtract)
            nc.vector.tensor_tensor(out=txr[:], in0=txr[:], in1=txr[:], op=OP.mult)
            wxs = tiny.tile([1, 160], f32, tag="wxs")
            nc.scalar.activation(out=wxs[:], in_=txr[:], func=AF.Exp, scale=-1.0 / SIG2)
            pwx = ps_s.tile([128, 160], f32, tag="pss")
            nc.tensor.matmul(pwx[:], cst["onesr"][:], wxs[:], start=True, stop=True)
            hwx128 = wpool.tile([128, 160], f32r, tag="hwx128")
            for n in range(2):
                nc.vector.tensor_scalar(out=hwx128[:, 80 * n:80 * n + 80],
                                        in0=pwx[:, 80 * n:80 * n + 80],
                                        scalar1=h2[:, n:n + 1], scalar2=None, op0=OP.mult)
            hwx = hpool.tile([2, C * W], f32r, tag="hwx")
            for n in range(2):
                nc.sync.dma_start(hwx[n:n + 1, :], hwx128[:, 80 * n:80 * n + 80])

            # ---- splat + residual add (in place on X) + store ----
            NCHUNK = 512
            for j in range(C * W // NCHUNK):
                sl = slice(j * NCHUNK, (j + 1) * NCHUNK)
                pp = ps_sp.tile([80, NCHUNK], f32, tag="pp")
                nc.tensor.matmul(pp[:], wyr[:], hwx[:, sl], start=True, stop=True)
                nc.vector.tensor_tensor(out=X[:, sl], in0=X[:, sl], in1=pp[:], op=OP.add)
            nc.sync.dma_start(out_d[b].rearrange("c h w -> h c w"), X[:])
        ctx.close()

    nc.compile()
    return nc


def kernel(**inputs):
    from concourse.bass_utils import run_bass_kernel_spmd

    x = np.asarray(inputs["x"], dtype=np.float32)
    masks = np.asarray(inputs["masks"], dtype=np.float32)
    wq = np.asarray(inputs["wq"], np.float32); wk = np.asarray(inputs["wk"], np.float32)
    wv = np.asarray(inputs["wv"], np.float32); wo = np.asarray(inputs["wo"], np.float32)
    w1 = np.asarray(inputs["w1"], np.float32); w2 = np.asarray(inputs["w2"], np.float32)

    consts = {
        "wqt": np.ascontiguousarray(wq.T), "wkt": np.ascontiguousarray(wk.T),
        "wvt": np.ascontiguousarray(wv.T), "wot": np.ascontiguousarray(wo.T),
        "w1t": np.ascontiguousarray(w1.T),
        "w2t0": np.ascontiguousarray(w2.T[0:128]), "w2t1": np.ascontiguousarray(w2.T[128:256]),
        "bq": np.asarray(inputs["bq"], np.float32).reshape(128, 1),
        "bk": np.asarray(inputs["bk"], np.float32).reshape(128, 1),
        "bv": np.asarray(inputs["bv"], np.float32).reshape(128, 1),
        "bo": np.asarray(inputs["bo"], np.float32).reshape(128, 1),
        "b1": np.ascontiguousarray(np.asarray(inputs["b1"], np.float32).reshape(2, 128).T),
        "b2": np.asarray(inputs["b2"], np.float32).reshape(128, 1),
        "g1": np.asarray(inputs["ln1_g"], np.float32).reshape(128, 1),
        "be1": np.asarray(inputs["ln1_b"], np.float32).reshape(128, 1),
        "g2": np.asarray(inputs["ln2_g"], np.float32).reshape(128, 1),
        "be2": np.asarray(inputs["ln2_b"], np.float32).reshape(128, 1),
        "ones80": np.ones((80, 1), np.float32), "ones128": np.ones((128, 1), np.float32),
        "id128": np.eye(128, dtype=np.float32),
        "ys80": np.arange(80, dtype=np.float32).reshape(80, 1),
        "xs2": np.tile(np.arange(80, dtype=np.float32), 2).reshape(1, 160),
        "yg2": np.tile(np.linspace(-1, 1, 80, dtype=np.float32), (2, 1)),
        "xg2": np.tile(np.linspace(-1, 1, 80, dtype=np.float32), (2, 1)),
        "signs2": np.array([[1.0], [-1.0]], np.float32),
        "onesr": np.ones((1, 128), np.float32),
        "xgxg": np.tile(np.linspace(-1, 1, 80, dtype=np.float32), 2).reshape(1, 160),
    }

    if "nc" not in _CACHE:
        _CACHE["nc"] = _build()
    nc = _CACHE["nc"]

    in_maps = []
    for c in range(N_CORES):
        m = {"x": np.ascontiguousarray(x[c * BL:(c + 1) * BL]),
             "masks": np.ascontiguousarray(masks[c * BL:(c + 1) * BL])}
        m.update(consts)
        in_maps.append(m)
    _CACHE["in_maps"] = in_maps
    res = run_bass_kernel_spmd(nc, in_maps, list(range(N_CORES))).results
    return np.concatenate([r["out"] for r in res], axis=0)

